# revision 65
# baseline (speedup 1.0000x reference)
"""Trainium2 Bass kernel for per-head-projection MHA + residual + LayerNorm.

Problem shapes (hardcoded): B=4, S=2048, E=512, H=8, DK=64, fp32.

Sharding: 8 cores, core c -> (batch b = c//2, query-half qh = c%2).
Each core computes the full transformer block for its 1024 query rows
(using the full 2048-row K/V of its batch), so per-core outputs are
disjoint slices of the final [4, 2048, 512] output and no collectives
are needed.

v2 design notes (driven by NTFF traces of v1, which showed the PE
HAM-throttled to 1.2 GHz for most of the run, f32r matmuls lowered to
the 1.5 cyc/row FP32-HIGH path, and a serialized scores->exp->PV
chain; measured 466 us -> 309 us):
  - every matmul operand is fp16 (weights converted host-side,
    activations produced in fp16 by the evacuation ops), so the whole
    pipeline runs the 1 cyc/row path and the PE stays HAM-warm.
  - attention inner loop is software-pipelined: PV(tt-1) is emitted
    after scores(tt)/exp(tt), so the PE always has scores work while
    the exp engines drain, and PV operands are always ready.
  - exp is split across two engines: even head of each pair uses the
    real ScalarE exp (fp16 out); odd head uses a Schraudolph fast-exp
    on VectorE (y_bits = int16(x*1024/ln2*scale + (15*1024 + 95)),
    truncating convert, bitcast fp16), ~1.7e-2 relative ctx error per
    approximated head, ~1e-3 end-to-end.
  - softmax denominators ride the PV matmul as a ones-column (M=65);
    normalize = ScalarE row copy -> fp16 PE broadcast -> VectorE
    reciprocal_approx on 64 lanes -> one mul per half into
    pair-stacked zT [128, SQ].
  - final linear contracts both heads of a pair per matmul (K=128).
  - Q natural tiles are kept in SBUF from the projection phase for the
    residual (no reload DMA); pair norms interleave with the next
    pair's first attention step.
"""

import sys

sys.path.insert(0, "/opt/trn_rl_repo")

import numpy as np

B, S, E, H, DK = 4, 2048, 512, 8, 64
NCORES = 8
SQ = (B * S) // NCORES  # 1024 query rows per core
HD = H * DK  # 512
PAIRS = H // 2
LN_EPS = 1e-5
SCALE = float(DK) ** -0.5
# Schraudolph fast-exp constants (fp16 bit pattern via int16 truncation)
SCH_A = float(np.float32(1024.0 / np.log(2.0) * SCALE))
SCH_B = float(np.float32(15.0 * 1024.0 + 95.0))

_PROGRAM_CACHE = {}


def _build_program(repeat=1):
    from contextlib import ExitStack

    import concourse.mybir as mybir
    import concourse.tile as tile
    from concourse import bacc
    dt = mybir.dt
    f32, f32r, f16 = dt.float32, dt.float32r, dt.float16
    i16 = dt.int16
    AF = mybir.ActivationFunctionType

    nc = bacc.Bacc("TRN2", target_bir_lowering=False, debug=False)

    # ---- DRAM I/O ----
    Qs_d = nc.dram_tensor("Qs", [SQ, E], f32, kind="ExternalInput").ap()
    Kf_d = nc.dram_tensor("Kf", [S, E], f32, kind="ExternalInput").ap()
    Vf_d = nc.dram_tensor("Vf", [S, E], f32, kind="ExternalInput").ap()
    Wq_d = nc.dram_tensor("Wq", [E, HD], f16, kind="ExternalInput").ap()
    Wk_d = nc.dram_tensor("Wk", [E, HD], f16, kind="ExternalInput").ap()
    Wv_d = nc.dram_tensor("Wv", [E, HD], f16, kind="ExternalInput").ap()
    Wf_d = nc.dram_tensor("Wf", [HD, E], f16, kind="ExternalInput").ap()
    bq_d = nc.dram_tensor("bq_t", [128, PAIRS], f32, kind="ExternalInput").ap()
    bk_d = nc.dram_tensor("bk_t", [128, PAIRS], f32, kind="ExternalInput").ap()
    bv_d = nc.dram_tensor("bv_t", [128, PAIRS], f16, kind="ExternalInput").ap()
    bf_d = nc.dram_tensor("bf_r", [1, E], f32, kind="ExternalInput").ap()
    ga_d = nc.dram_tensor("gamma_r", [1, E], f32r, kind="ExternalInput").ap()
    be_d = nc.dram_tensor("beta_r", [1, E], f32r, kind="ExternalInput").ap()
    id_d = nc.dram_tensor("ident", [128, 128], f32r, kind="ExternalInput").ap()
    Out_d = nc.dram_tensor("Out", [SQ, E], f32, kind="ExternalOutput").ap()

    with tile.TileContext(nc) as tc:
        for rep in range(repeat):
            _emit_body(
                nc, tc, ExitStack, mybir, f32, f32r, f16, i16, AF,
                Qs_d, Kf_d, Vf_d, Wq_d, Wk_d, Wv_d, Wf_d, bq_d, bk_d, bv_d,
                bf_d, ga_d, be_d, id_d, Out_d, rep,
            )

    nc.compile()
    return nc


def _emit_body(
    nc, tc, ExitStack, mybir, f32, f32r, f16, i16, AF,
    Qs_d, Kf_d, Vf_d, Wq_d, Wk_d, Wv_d, Wf_d, bq_d, bk_d, bv_d,
    bf_d, ga_d, be_d, id_d, Out_d, rep,
):
    Alu = mybir.AluOpType
    with ExitStack() as ctx:
        const_p = ctx.enter_context(tc.tile_pool(name="const", bufs=1))
        w_p = ctx.enter_context(tc.tile_pool(name="weights", bufs=1))
        act_p = ctx.enter_context(tc.tile_pool(name="acts", bufs=1))
        xt_p = ctx.enter_context(tc.tile_pool(name="xt", bufs=4))
        nat_p = ctx.enter_context(tc.tile_pool(name="nat", bufs=4))
        exa_p = ctx.enter_context(tc.tile_pool(name="exa", bufs=4))
        exd_p = ctx.enter_context(tc.tile_pool(name="exd", bufs=3))
        rd_p = ctx.enter_context(tc.tile_pool(name="rden", bufs=2))
        ln_p = ctx.enter_context(tc.tile_pool(name="ln", bufs=3))
        st_p = ctx.enter_context(tc.tile_pool(name="stats", bufs=4))

        # ---------- constants & weights ----------
        ident = const_p.tile([128, 128], f32r)
        nc.sync.dma_start(ident[:], id_d[:])
        ident_h = const_p.tile([128, 128], f16)
        nc.scalar.copy(ident_h[:], ident[:])
        ones_t = const_p.tile([128, 128], f32r)
        nc.vector.memset(ones_t[:].bitcast(f32), 1.0)
        eps_t = const_p.tile([128, 1], f32)
        nc.vector.memset(eps_t[:], LN_EPS)

        wq = [w_p.tile([128, HD], f16, tag=f"wq{i}", name=f"wq{i}_{rep}") for i in range(4)]
        wk = [w_p.tile([128, HD], f16, tag=f"wk{i}", name=f"wk{i}_{rep}") for i in range(4)]
        wv = [w_p.tile([128, HD], f16, tag=f"wv{i}", name=f"wv{i}_{rep}") for i in range(4)]
        # final-linear weights pair-stacked: wfp[p] = Wf rows for heads 2p,2p+1
        wfp = [w_p.tile([128, E], f16, tag=f"wfp{p}", name=f"wfp{p}_{rep}") for p in range(PAIRS)]
        bq_t = const_p.tile([128, PAIRS], f32)
        bk_t = const_p.tile([128, PAIRS], f32)
        bv_t = const_p.tile([128, PAIRS], f16)
        bf_r = const_p.tile([1, E], f32)
        ga_r = const_p.tile([1, E], f32r)
        be_r = const_p.tile([1, E], f32r)
        bfe_sb = const_p.tile([1, E], f16)
        ones_h = const_p.tile([1, 128], f16)
        nc.vector.memset(ones_h[:], 1.0)
        gab = act_p.tile([128, E], f32, tag="gab")
        beb = act_p.tile([128, E], f32, tag="beb")
        # write-only dump for Square's unused tensor output (only accum_out
        # matters); keeps the real xn free of a cross-engine WAW hazard
        dead = act_p.tile([128, E], f32, tag="dead")

        def load_weights_q():
            for ec in range(4):
                nc.sync.dma_start(wq[ec][:], Wq_d[ec * 128 : (ec + 1) * 128, :])
            nc.sync.dma_start(bq_t[:], bq_d[:])

        def load_weights_k():
            for ec in range(4):
                nc.sync.dma_start(wk[ec][:], Wk_d[ec * 128 : (ec + 1) * 128, :])
            nc.sync.dma_start(bk_t[:], bk_d[:])

        def load_weights_rest():
            for ec in range(4):
                nc.sync.dma_start(wv[ec][:], Wv_d[ec * 128 : (ec + 1) * 128, :])
            for p in range(PAIRS):
                nc.sync.dma_start(wfp[p][:], Wf_d[p * 128 : (p + 1) * 128, :])
            nc.sync.dma_start(bv_t[:], bv_d[:])
            nc.sync.dma_start(bf_r[:], bf_d[:])
            nc.sync.dma_start(ga_r[:], ga_d[:])
            nc.sync.dma_start(be_r[:], be_d[:])

        def emit_pre(pre_ps):
            # bf_eff = bf + bv @ Wf (bv folds through the final linear since
            # softmax rows sum to 1); broadcast gamma/beta to 128 partitions
            # via PE outer products with a ones column.
            bfe_ps = pre_ps.tile([1, E], f32, tag="bfe", bufs=1)
            for p in range(PAIRS):
                nc.tensor.matmul(
                    bfe_ps[:], bv_t[:, p : p + 1], wfp[p][:],
                    start=(p == 0), stop=(p == PAIRS - 1),
                )
            with nc.allow_low_precision(reason="bf_eff folded bias, fp16"):
                nc.vector.tensor_add(bfe_sb[:], bfe_ps[:], bf_r[:])
            for row, dst in ((ga_r, gab), (be_r, beb)):
                bc_ps = pre_ps.tile([128, E], f32, tag="bc", bufs=1)
                nc.tensor.matmul(
                    bc_ps[:], ones_t[0:1, :], row[:], start=True, stop=True
                )
                nc.vector.tensor_copy(dst[:], bc_ps[:])
            # warm the ACT exp table before the attention phase needs it
            warm = st_p.tile([1, 1], f16, tag="warm")
            nc.scalar.activation(warm[:], eps_t[0:1, 0:1], AF.Exp)

        # ---------- persistent activations (all fp16 so every attention
        # matmul runs the 1-cycle/row fp16 path) ----------
        qT = [act_p.tile([128, SQ], f16, tag=f"qT{i}", name=f"qT{i}_{rep}") for i in range(PAIRS)]
        kT = [act_p.tile([128, S], f16, tag=f"kT{i}", name=f"kT{i}_{rep}") for i in range(PAIRS)]
        v_aug = [act_p.tile([128, H * (DK + 1)], f16, tag=f"vaug{i}", name=f"vaug{i}_{rep}") for i in range(16)]
        # zT pair-stacked: heads 2p (parts 0-63) and 2p+1 (parts 64-127)
        zT = [act_p.tile([128, SQ], f16, tag=f"zT{p}", name=f"zT{p}_{rep}") for p in range(PAIRS)]
        # natural Q rows kept for the residual: tile j holds rows j*256..j*256+255
        # folded as [128, 2E] (cols 0:E -> rows +0..127, cols E:2E -> rows +128..255)
        qnat = [act_p.tile([128, 2 * E], f32r, tag=f"qn{j}", name=f"qn{j}_{rep}") for j in range(4)]

        # ---------- streamed transpose + projection ----------
        def load_chunkT(src_dram, s0, xtiles, tp_ps, keep=None):
            """Load 512 natural rows [s0:s0+512] as two [128, 2E] folded fp16
            tiles (K/V: GpSimd cast-DMA straight from fp32 HBM -> half the
            bytes; Q: fp32 DMA kept for the residual + ScalarE fp16 copy),
            then PE-transpose (fp16, 1 cyc/row) into xtiles and evacuate via
            VectorE 2x-rate copies."""
            nats = []
            for half in range(2):
                r0 = s0 + half * 256
                srcv = src_dram[r0 : r0 + 256, :]
                nath = nat_p.tile([128, 2 * E], f16, tag="nath", name=f"nh{s0}_{half}_{rep}")
                natt = keep[half] if keep is not None else nat_p.tile(
                    [128, 2 * E], f32r, tag="nat", name=f"nf{s0}_{half}_{rep}"
                )
                dst = natt[:].rearrange("p (sb e) -> p sb e", sb=2, e=E)
                nc.sync.dma_start(
                    dst, srcv.bitcast(f32r).rearrange("(sb p) e -> p sb e", sb=2, p=128)
                )
                nc.scalar.copy(nath[:], natt[:])
                nats.append(nath)
            for eh in range(2):  # ec pair: (2*eh, 2*eh+1)
                tp = tp_ps.tile([128, 1024], f16, tag="tp", name=f"tp_{s0}_{eh}_{rep}")
                for sub in range(2):
                    ec = 2 * eh + sub
                    for st in range(4):
                        nc.tensor.transpose(
                            tp[:, sub * 512 + st * 128 : sub * 512 + (st + 1) * 128],
                            nats[st // 2][:, (st % 2) * E + ec * 128 : (st % 2) * E + (ec + 1) * 128],
                            ident_h[:],
                        )
                nc.vector.tensor_copy(xtiles[eh][:], tp[:])

        with (
            tc.tile_pool(name="psum_tp", bufs=2, space="PSUM") as tp_ps,
            tc.tile_pool(name="psum_proj", bufs=4, space="PSUM") as proj_ps,
        ):
            # Q -> qT pairs; stash natural tiles for the residual
            for sc in range(SQ // 512):
                qx = [xt_p.tile([128, 1024], f16, tag="xt", name=f"qx{sc}_{i}_{rep}") for i in range(2)]
                load_chunkT(Qs_d, sc * 512, qx, tp_ps, keep=qnat[2 * sc : 2 * sc + 2])
                if sc == 0:
                    load_weights_q()
                for p in range(PAIRS):
                    pr = proj_ps.tile([128, 512], f32, tag="proj")
                    for ec in range(4):
                        nc.tensor.matmul(
                            pr[:], wq[ec][:, p * 128 : (p + 1) * 128],
                            qx[ec // 2][:, (ec % 2) * 512 : (ec % 2) * 512 + 512],
                            start=(ec == 0), stop=(ec == 3),
                        )
                    nc.scalar.activation(
                        qT[p][:, sc * 512 : (sc + 1) * 512], pr[:], AF.Identity,
                        bias=bq_t[:, p : p + 1],
                    )
            # K -> kT pairs
            for sc in range(S // 512):
                kx = [xt_p.tile([128, 1024], f16, tag="xt", name=f"kx{sc}_{i}_{rep}") for i in range(2)]
                load_chunkT(Kf_d, sc * 512, kx, tp_ps)
                if sc == 0:
                    load_weights_k()
                elif sc == 2:
                    load_weights_rest()
                for p in range(PAIRS):
                    pr = proj_ps.tile([128, 512], f32, tag="proj")
                    for ec in range(4):
                        nc.tensor.matmul(
                            pr[:], wk[ec][:, p * 128 : (p + 1) * 128],
                            kx[ec // 2][:, (ec % 2) * 512 : (ec % 2) * 512 + 512],
                            start=(ec == 0), stop=(ec == 3),
                        )
                    nc.scalar.activation(
                        kT[p][:, sc * 512 : (sc + 1) * 512], pr[:], AF.Identity,
                        bias=bk_t[:, p : p + 1],
                    )
            # V -> v_aug (natural [seq, hd] with a ones column per head)
            for sc in range(S // 512):
                vx = [xt_p.tile([128, 1024], f16, tag="xt", name=f"vx{sc}_{i}_{rep}") for i in range(2)]
                load_chunkT(Vf_d, sc * 512, vx, tp_ps)
                for tl in range(4):
                    tt = sc * 4 + tl
                    pr = proj_ps.tile([128, 512], f32, tag="proj")
                    for ec in range(4):
                        nc.tensor.matmul(
                            pr[:],
                            vx[ec // 2][:, (ec % 2) * 512 + tl * 128 : (ec % 2) * 512 + (tl + 1) * 128],
                            wv[ec][:],
                            start=(ec == 0), stop=(ec == 3),
                        )
                    va3 = v_aug[tt][:].rearrange("p (h x) -> p h x", h=H, x=DK + 1)
                    pr3 = pr[:].rearrange("p (h d) -> p h d", h=H, d=DK)
                    nc.vector.tensor_copy(va3[:, :, 0:DK], pr3)
                    nc.vector.memset(va3[:, :, DK : DK + 1], 1.0)

        # bf_eff / gamma / beta broadcasts + exp-table warmup, in their own
        # small PSUM scope between the projection and attention phases
        with tc.tile_pool(name="psum_pre", bufs=1, space="PSUM") as pre_ps:
            emit_pre(pre_ps)

        # ---------- attention: software-pipelined scores/exp/PV ----------
        # Even head of pair lives on partitions 0-63, odd on 64-127 of the
        # qT/kT pair tiles -> the two K=64 scores matmuls of a pair run on
        # disjoint PE row-groups concurrently.
        with (
            tc.tile_pool(name="psum_sc", bufs=2, space="PSUM") as sc_ps_p,
            tc.tile_pool(name="psum_pv", bufs=2, space="PSUM") as pv_ps_p,
        ):
            pvs_all = [None] * PAIRS
            exs_all = [[None] * 16 for _ in range(PAIRS)]

            def emit_scores_exp(p, tt):
                scs = [
                    sc_ps_p.tile([128, SQ], f32, tag="sc", name=f"sc{p}_{tt}_{h}_{rep}")
                    for h in range(2)
                ]
                # interleave halves so adjacent matmuls sit on disjoint PE
                # row-groups (0-63 vs 64-127) and stream concurrently
                for qc in range(SQ // 512):
                    for half in range(2):
                        pb = 64 * half
                        nc.tensor.matmul(
                            scs[half][:, qc * 512 : (qc + 1) * 512],
                            kT[p][pb : pb + DK, tt * 128 : (tt + 1) * 128],
                            qT[p][pb : pb + DK, qc * 512 : (qc + 1) * 512],
                            start=True, stop=True,
                        )
                # half0 -> ScalarE true exp; half1 -> VectorE fast-exp.
                # At tt 0/8 (away from the pair boundary, where ScalarE does
                # the denominator copies) half1 also goes to ScalarE so the
                # DVE keeps headroom for the previous pair's norm.
                exa = exa_p.tile([128, SQ], f16, tag="exa", name=f"exa{p}_{tt}_{rep}")
                nc.scalar.activation(exa[:], scs[0][:], AF.Exp, scale=SCALE)
                if tt in (0, 8):
                    exd = exa_p.tile([128, SQ], f16, tag="exa", name=f"exd{p}_{tt}_{rep}")
                    nc.scalar.activation(exd[:], scs[1][:], AF.Exp, scale=SCALE)
                    exs_all[p][tt] = (exa, exd[:])
                else:
                    exd = exd_p.tile([128, SQ], i16, tag="exd", name=f"exd{p}_{tt}_{rep}")
                    nc.vector.tensor_scalar(
                        exd[:], scs[1][:], SCH_A, SCH_B, Alu.mult, Alu.add
                    )
                    exs_all[p][tt] = (exa, exd[:].bitcast(f16))

            def emit_pv(p, tt):
                exa, exd = exs_all[p][tt]
                pvs = pvs_all[p]
                for half, ex in ((0, exa[:]), (1, exd)):
                    h = 2 * p + half
                    for qc in range(SQ // 512):
                        nc.tensor.matmul(
                            pvs[half][:, qc * 512 : (qc + 1) * 512],
                            v_aug[tt][:, h * (DK + 1) : (h + 1) * (DK + 1)],
                            ex[:, qc * 512 : (qc + 1) * 512],
                            start=(tt == 0), stop=(tt == 15),
                        )
                exs_all[p][tt] = None

            def emit_norm(p):
                # normalize: ScalarE copies the raw denominator row to SBUF
                # (fp16), PE broadcasts it down 64 partitions, VectorE takes
                # the reciprocal on 64 lanes, then one mul per half into the
                # pair-stacked zT
                pvs = pvs_all[p]
                for half in range(2):
                    den = rd_p.tile([1, SQ], f16, tag="rd", name=f"rd{p}_{half}_{rep}")
                    nc.scalar.copy(den[:], pvs[half][DK : DK + 1, :])
                    rb = sc_ps_p.tile([128, SQ], f32, tag="sc", name=f"rb{p}_{half}_{rep}")
                    for qc in range(SQ // 512):
                        nc.tensor.matmul(
                            rb[0:DK, qc * 512 : (qc + 1) * 512],
                            ones_h[:, 0:DK],
                            den[:, qc * 512 : (qc + 1) * 512],
                            start=True, stop=True,
                        )
                    rb_sb = rd_p.tile([DK, SQ], f32, tag="rb", name=f"rb{p}_{half}_{rep}")
                    nc.vector.reciprocal_approx_fast(rb_sb[:], rb[0:DK, :])
                    with nc.allow_low_precision(reason="fp16 attention ctx"):
                        nc.vector.tensor_mul(
                            zT[p][64 * half : 64 * half + DK, :],
                            pvs[half][0:DK, :],
                            rb_sb[:],
                        )
                pvs_all[p] = None

            for g in range(PAIRS * 16 + 1):
                if g < PAIRS * 16:
                    p, tt = divmod(g, 16)
                    if tt == 0:
                        pvs_all[p] = [
                            pv_ps_p.tile(
                                [DK + 1, SQ], f32, tag="pv", name=f"pv{p}_{h}_{rep}"
                            )
                            for h in range(2)
                        ]
                    emit_scores_exp(p, tt)
                if g > 0:
                    pp, ptt = divmod(g - 1, 16)
                    emit_pv(pp, ptt)
                    if ptt == 15:
                        emit_norm(pp)

        # ---------- final linear + residual + LayerNorm ----------
        with tc.tile_pool(name="psum_f", bufs=4, space="PSUM") as f_ps_p:
            for qb in range(SQ // 128):
                f_ps = f_ps_p.tile([128, E], f32, tag="f")
                for p in range(PAIRS):
                    nc.tensor.matmul(
                        f_ps[:], zT[p][:, qb * 128 : (qb + 1) * 128], wfp[p][:],
                        start=(p == 0), stop=False,
                    )
                nc.tensor.matmul(
                    f_ps[:], ones_h[:, 0:128], bfe_sb[:],
                    start=False, stop=True,
                )
                qn = qnat[qb // 2][:, (qb % 2) * E : (qb % 2) * E + E].bitcast(f32)
                x = ln_p.tile([128, E], f32, tag="x")
                nm = st_p.tile([128, 1], f32, tag="nm")
                nc.vector.scalar_tensor_tensor(
                    x[:], f_ps[:], 1.0, qn,
                    mybir.AluOpType.mult, mybir.AluOpType.add,
                    accum_out=nm[:],
                )
                nc.vector.tensor_scalar_mul(nm[:], nm[:], -1.0 / E)
                xn = ln_p.tile([128, E], f32, tag="xn")
                ss = st_p.tile([128, 1], f32, tag="ss")
                nc.scalar.activation(dead[:], x[:], AF.Square, accum_out=ss[:])
                # var = E[x^2] - mu^2; bias for sqrt = eps - mu^2
                vb = st_p.tile([128, 1], f32, tag="vb")
                nc.vector.scalar_tensor_tensor(
                    vb[:], nm[:], -1.0, nm[:],
                    mybir.AluOpType.mult, mybir.AluOpType.mult,
                )
                nc.vector.tensor_add(vb[:], vb[:], eps_t[:])
                sd = st_p.tile([128, 1], f32, tag="sd")
                nc.scalar.activation(
                    sd[:], ss[:], AF.Sqrt, bias=vb[:, 0:1], scale=1.0 / E
                )
                rstd = st_p.tile([128, 1], f32, tag="rstd")
                nc.vector.reciprocal(rstd[:], sd[:])
                nmr = st_p.tile([128, 1], f32, tag="nmr")
                nc.vector.tensor_mul(nmr[:], nm[:], rstd[:])
                nc.vector.tensor_scalar(
                    xn[:], x[:], rstd[:, 0:1], nmr[:, 0:1], Alu.mult, Alu.add
                )
                # gamma on GpSimd (slow but mid-chain, overlaps other qbs);
                # beta on VectorE so the store isn't gated on a 1.3us Pool op
                nc.gpsimd.tensor_tensor(xn[:], xn[:], gab[:], mybir.AluOpType.mult)
                nc.vector.tensor_add(xn[:], xn[:], beb[:])
                nc.sync.dma_start(Out_d[qb * 128 : (qb + 1) * 128, :], xn[:])


def _get_program(repeat=1):
    key = f"nc{repeat}"
    if key not in _PROGRAM_CACHE:
        _PROGRAM_CACHE[key] = _build_program(repeat)
    return _PROGRAM_CACHE[key]


def _make_in_maps(Q, K, V, Wq, bq, Wk, bk, Wv, bv, Wf, bf, gamma, beta):
    f32 = np.float32

    f16 = np.float16

    def per_head_w(W):  # [H, E, DK] -> [E, H*DK], fp16
        return np.ascontiguousarray(W.transpose(1, 0, 2).reshape(E, HD), dtype=f16)

    def pair_bias(b, dt=f32):  # [H, DK] -> [128, PAIRS]; partition = (h%2)*64 + d
        return np.ascontiguousarray(
            b.reshape(PAIRS, 2, DK).transpose(1, 2, 0).reshape(128, PAIRS), dtype=dt
        )

    Wq_r, Wk_r, Wv_r = per_head_w(Wq), per_head_w(Wk), per_head_w(Wv)
    bq_r, bk_r = pair_bias(bq), pair_bias(bk)
    bv_r = pair_bias(bv, f16)
    Wf_c = np.ascontiguousarray(Wf, dtype=f16)
    bf_r = np.ascontiguousarray(bf.reshape(1, E), dtype=f32)
    ga_r = np.ascontiguousarray(gamma.reshape(1, E), dtype=f32)
    be_r = np.ascontiguousarray(beta.reshape(1, E), dtype=f32)

    in_maps = []
    for c in range(NCORES):
        b, qh = c // 2, c % 2
        in_maps.append(
            {
                "Qs": np.ascontiguousarray(Q[b, qh * SQ : (qh + 1) * SQ], dtype=f32),
                "Kf": np.ascontiguousarray(K[b], dtype=f32),
                "Vf": np.ascontiguousarray(V[b], dtype=f32),
                "Wq": Wq_r,
                "Wk": Wk_r,
                "Wv": Wv_r,
                "Wf": Wf_c,
                "bq_t": bq_r,
                "bk_t": bk_r,
                "bv_t": bv_r,
                "bf_r": bf_r,
                "gamma_r": ga_r,
                "beta_r": be_r,
                "ident": np.eye(128, dtype=f32),
            }
        )
    return in_maps


def run_spmd(in_maps, **kwargs):
    from concourse.bass_utils import run_bass_kernel_spmd

    nc = _get_program()
    return run_bass_kernel_spmd(nc, in_maps, list(range(NCORES)), **kwargs)


def kernel(**inputs) -> np.ndarray:
    in_maps = _make_in_maps(**inputs)
    res = run_spmd(in_maps)
    out = np.empty((B, S, E), np.float32)
    for c in range(NCORES):
        b, qh = c // 2, c % 2
        out[b, qh * SQ : (qh + 1) * SQ, :] = res.results[c]["Out"]
    return out


if __name__ == "__main__":
    import time

    t0 = time.time()
    _get_program()
    print(f"built ok in {time.time() - t0:.1f}s")


# revision 68
# speedup vs baseline: 1.0030x; 1.0030x over previous
"""Trainium2 Bass kernel for per-head-projection MHA + residual + LayerNorm.

Problem shapes (hardcoded): B=4, S=2048, E=512, H=8, DK=64, fp32.

Sharding: 8 cores, core c -> (batch b = c//2, query-half qh = c%2).
Each core computes the full transformer block for its 1024 query rows
(using the full 2048-row K/V of its batch), so per-core outputs are
disjoint slices of the final [4, 2048, 512] output and no collectives
are needed.

v2 design notes (driven by NTFF traces of v1, which showed the PE
HAM-throttled to 1.2 GHz for most of the run, f32r matmuls lowered to
the 1.5 cyc/row FP32-HIGH path, and a serialized scores->exp->PV
chain; measured 466 us -> 309 us):
  - every matmul operand is fp16 (weights converted host-side,
    activations produced in fp16 by the evacuation ops), so the whole
    pipeline runs the 1 cyc/row path and the PE stays HAM-warm.
  - attention inner loop is software-pipelined: PV(tt-1) is emitted
    after scores(tt)/exp(tt), so the PE always has scores work while
    the exp engines drain, and PV operands are always ready.
  - exp is split across two engines: even head of each pair uses the
    real ScalarE exp (fp16 out); odd head uses a Schraudolph fast-exp
    on VectorE (y_bits = int16(x*1024/ln2*scale + (15*1024 + 95)),
    truncating convert, bitcast fp16), ~1.7e-2 relative ctx error per
    approximated head, ~1e-3 end-to-end.
  - softmax denominators ride the PV matmul as a ones-column (M=65);
    normalize = ScalarE row copy -> fp16 PE broadcast -> VectorE
    reciprocal_approx on 64 lanes -> one mul per half into
    pair-stacked zT [128, SQ].
  - final linear contracts both heads of a pair per matmul (K=128).
  - Q natural tiles are kept in SBUF from the projection phase for the
    residual (no reload DMA); pair norms interleave with the next
    pair's first attention step.
"""

import sys

sys.path.insert(0, "/opt/trn_rl_repo")

import numpy as np

B, S, E, H, DK = 4, 2048, 512, 8, 64
NCORES = 8
SQ = (B * S) // NCORES  # 1024 query rows per core
HD = H * DK  # 512
PAIRS = H // 2
LN_EPS = 1e-5
SCALE = float(DK) ** -0.5
# Schraudolph fast-exp constants (fp16 bit pattern via int16 truncation)
SCH_A = float(np.float32(1024.0 / np.log(2.0) * SCALE))
SCH_B = float(np.float32(15.0 * 1024.0 + 95.0))

_PROGRAM_CACHE = {}


def _build_program(repeat=1):
    from contextlib import ExitStack

    import concourse.mybir as mybir
    import concourse.tile as tile
    from concourse import bacc
    dt = mybir.dt
    f32, f32r, f16 = dt.float32, dt.float32r, dt.float16
    i16 = dt.int16
    AF = mybir.ActivationFunctionType

    nc = bacc.Bacc("TRN2", target_bir_lowering=False, debug=False)

    # ---- DRAM I/O ----
    Qs_d = nc.dram_tensor("Qs", [SQ, E], f32, kind="ExternalInput").ap()
    Kf_d = nc.dram_tensor("Kf", [S, E], f32, kind="ExternalInput").ap()
    Vf_d = nc.dram_tensor("Vf", [S, E], f32, kind="ExternalInput").ap()
    Wq_d = nc.dram_tensor("Wq", [E, HD], f16, kind="ExternalInput").ap()
    Wk_d = nc.dram_tensor("Wk", [E, HD], f16, kind="ExternalInput").ap()
    Wv_d = nc.dram_tensor("Wv", [E, HD], f16, kind="ExternalInput").ap()
    Wf_d = nc.dram_tensor("Wf", [HD, E], f16, kind="ExternalInput").ap()
    bq_d = nc.dram_tensor("bq_t", [128, PAIRS], f32, kind="ExternalInput").ap()
    bk_d = nc.dram_tensor("bk_t", [128, PAIRS], f32, kind="ExternalInput").ap()
    bv_d = nc.dram_tensor("bv_t", [128, PAIRS], f16, kind="ExternalInput").ap()
    bf_d = nc.dram_tensor("bf_r", [1, E], f32, kind="ExternalInput").ap()
    ga_d = nc.dram_tensor("gamma_r", [1, E], f32r, kind="ExternalInput").ap()
    be_d = nc.dram_tensor("beta_r", [1, E], f32r, kind="ExternalInput").ap()
    id_d = nc.dram_tensor("ident", [128, 128], f32r, kind="ExternalInput").ap()
    Out_d = nc.dram_tensor("Out", [SQ, E], f32, kind="ExternalOutput").ap()

    with tile.TileContext(nc) as tc:
        for rep in range(repeat):
            _emit_body(
                nc, tc, ExitStack, mybir, f32, f32r, f16, i16, AF,
                Qs_d, Kf_d, Vf_d, Wq_d, Wk_d, Wv_d, Wf_d, bq_d, bk_d, bv_d,
                bf_d, ga_d, be_d, id_d, Out_d, rep,
            )

    nc.compile()
    return nc


def _emit_body(
    nc, tc, ExitStack, mybir, f32, f32r, f16, i16, AF,
    Qs_d, Kf_d, Vf_d, Wq_d, Wk_d, Wv_d, Wf_d, bq_d, bk_d, bv_d,
    bf_d, ga_d, be_d, id_d, Out_d, rep,
):
    Alu = mybir.AluOpType
    with ExitStack() as ctx:
        const_p = ctx.enter_context(tc.tile_pool(name="const", bufs=1))
        w_p = ctx.enter_context(tc.tile_pool(name="weights", bufs=1))
        act_p = ctx.enter_context(tc.tile_pool(name="acts", bufs=1))
        xt_p = ctx.enter_context(tc.tile_pool(name="xt", bufs=4))
        nat_p = ctx.enter_context(tc.tile_pool(name="nat", bufs=4))
        exa_p = ctx.enter_context(tc.tile_pool(name="exa", bufs=4))
        exd_p = ctx.enter_context(tc.tile_pool(name="exd", bufs=3))
        rd_p = ctx.enter_context(tc.tile_pool(name="rden", bufs=2))
        ln_p = ctx.enter_context(tc.tile_pool(name="ln", bufs=3))
        st_p = ctx.enter_context(tc.tile_pool(name="stats", bufs=4))

        # ---------- constants & weights ----------
        ident = const_p.tile([128, 128], f32r)
        nc.sync.dma_start(ident[:], id_d[:])
        ident_h = const_p.tile([128, 128], f16)
        nc.scalar.copy(ident_h[:], ident[:])
        ones_t = const_p.tile([128, 128], f32r)
        nc.vector.memset(ones_t[:].bitcast(f32), 1.0)
        eps_t = const_p.tile([128, 1], f32)
        nc.vector.memset(eps_t[:], LN_EPS)

        wq = [w_p.tile([128, HD], f16, tag=f"wq{i}", name=f"wq{i}_{rep}") for i in range(4)]
        wk = [w_p.tile([128, HD], f16, tag=f"wk{i}", name=f"wk{i}_{rep}") for i in range(4)]
        wv = [w_p.tile([128, HD], f16, tag=f"wv{i}", name=f"wv{i}_{rep}") for i in range(4)]
        # final-linear weights pair-stacked: wfp[p] = Wf rows for heads 2p,2p+1
        wfp = [w_p.tile([128, E], f16, tag=f"wfp{p}", name=f"wfp{p}_{rep}") for p in range(PAIRS)]
        bq_t = const_p.tile([128, PAIRS], f32)
        bk_t = const_p.tile([128, PAIRS], f32)
        bv_t = const_p.tile([128, PAIRS], f16)
        bf_r = const_p.tile([1, E], f32)
        ga_r = const_p.tile([1, E], f32r)
        be_r = const_p.tile([1, E], f32r)
        bfe_sb = const_p.tile([1, E], f16)
        ones_h = const_p.tile([1, 128], f16)
        nc.vector.memset(ones_h[:], 1.0)
        gab = act_p.tile([128, E], f32, tag="gab")
        beb = act_p.tile([128, E], f32, tag="beb")

        def load_weights_q():
            for ec in range(4):
                nc.sync.dma_start(wq[ec][:], Wq_d[ec * 128 : (ec + 1) * 128, :])
            nc.sync.dma_start(bq_t[:], bq_d[:])

        def load_weights_k():
            for ec in range(4):
                nc.sync.dma_start(wk[ec][:], Wk_d[ec * 128 : (ec + 1) * 128, :])
            nc.sync.dma_start(bk_t[:], bk_d[:])

        def load_weights_rest():
            for ec in range(4):
                nc.sync.dma_start(wv[ec][:], Wv_d[ec * 128 : (ec + 1) * 128, :])
            for p in range(PAIRS):
                nc.sync.dma_start(wfp[p][:], Wf_d[p * 128 : (p + 1) * 128, :])
            nc.sync.dma_start(bv_t[:], bv_d[:])
            nc.sync.dma_start(bf_r[:], bf_d[:])
            nc.sync.dma_start(ga_r[:], ga_d[:])
            nc.sync.dma_start(be_r[:], be_d[:])

        def emit_pre(pre_ps):
            # bf_eff = bf + bv @ Wf (bv folds through the final linear since
            # softmax rows sum to 1); broadcast gamma/beta to 128 partitions
            # via PE outer products with a ones column.
            bfe_ps = pre_ps.tile([1, E], f32, tag="bfe", bufs=1)
            for p in range(PAIRS):
                nc.tensor.matmul(
                    bfe_ps[:], bv_t[:, p : p + 1], wfp[p][:],
                    start=(p == 0), stop=(p == PAIRS - 1),
                )
            with nc.allow_low_precision(reason="bf_eff folded bias, fp16"):
                nc.vector.tensor_add(bfe_sb[:], bfe_ps[:], bf_r[:])
            for row, dst in ((ga_r, gab), (be_r, beb)):
                bc_ps = pre_ps.tile([128, E], f32, tag="bc", bufs=1)
                nc.tensor.matmul(
                    bc_ps[:], ones_t[0:1, :], row[:], start=True, stop=True
                )
                nc.vector.tensor_copy(dst[:], bc_ps[:])
            # warm the ACT exp table before the attention phase needs it
            warm = st_p.tile([1, 1], f16, tag="warm")
            nc.scalar.activation(warm[:], eps_t[0:1, 0:1], AF.Exp)

        # ---------- persistent activations (all fp16 so every attention
        # matmul runs the 1-cycle/row fp16 path) ----------
        qT = [act_p.tile([128, SQ], f16, tag=f"qT{i}", name=f"qT{i}_{rep}") for i in range(PAIRS)]
        kT = [act_p.tile([128, S], f16, tag=f"kT{i}", name=f"kT{i}_{rep}") for i in range(PAIRS)]
        v_aug = [act_p.tile([128, H * (DK + 1)], f16, tag=f"vaug{i}", name=f"vaug{i}_{rep}") for i in range(16)]
        # zT pair-stacked: heads 2p (parts 0-63) and 2p+1 (parts 64-127)
        zT = [act_p.tile([128, SQ], f16, tag=f"zT{p}", name=f"zT{p}_{rep}") for p in range(PAIRS)]
        # natural Q rows kept for the residual: tile j holds rows j*256..j*256+255
        # folded as [128, 2E] (cols 0:E -> rows +0..127, cols E:2E -> rows +128..255)
        qnat = [act_p.tile([128, 2 * E], f32r, tag=f"qn{j}", name=f"qn{j}_{rep}") for j in range(4)]

        # ---------- streamed transpose + projection ----------
        def load_chunkT(src_dram, s0, xtiles, tp_ps, keep=None):
            """Load 512 natural rows [s0:s0+512] as two [128, 2E] folded fp16
            tiles (K/V: GpSimd cast-DMA straight from fp32 HBM -> half the
            bytes; Q: fp32 DMA kept for the residual + ScalarE fp16 copy),
            then PE-transpose (fp16, 1 cyc/row) into xtiles and evacuate via
            VectorE 2x-rate copies."""
            nats = []
            for half in range(2):
                r0 = s0 + half * 256
                srcv = src_dram[r0 : r0 + 256, :]
                nath = nat_p.tile([128, 2 * E], f16, tag="nath", name=f"nh{s0}_{half}_{rep}")
                natt = keep[half] if keep is not None else nat_p.tile(
                    [128, 2 * E], f32r, tag="nat", name=f"nf{s0}_{half}_{rep}"
                )
                dst = natt[:].rearrange("p (sb e) -> p sb e", sb=2, e=E)
                nc.sync.dma_start(
                    dst, srcv.bitcast(f32r).rearrange("(sb p) e -> p sb e", sb=2, p=128)
                )
                nc.scalar.copy(nath[:], natt[:])
                nats.append(nath)
            for eh in range(2):  # ec pair: (2*eh, 2*eh+1)
                tp = tp_ps.tile([128, 1024], f16, tag="tp", name=f"tp_{s0}_{eh}_{rep}")
                for sub in range(2):
                    ec = 2 * eh + sub
                    for st in range(4):
                        nc.tensor.transpose(
                            tp[:, sub * 512 + st * 128 : sub * 512 + (st + 1) * 128],
                            nats[st // 2][:, (st % 2) * E + ec * 128 : (st % 2) * E + (ec + 1) * 128],
                            ident_h[:],
                        )
                nc.vector.tensor_copy(xtiles[eh][:], tp[:])

        with (
            tc.tile_pool(name="psum_tp", bufs=2, space="PSUM") as tp_ps,
            tc.tile_pool(name="psum_proj", bufs=4, space="PSUM") as proj_ps,
        ):
            # Q -> qT pairs; stash natural tiles for the residual
            for sc in range(SQ // 512):
                qx = [xt_p.tile([128, 1024], f16, tag="xt", name=f"qx{sc}_{i}_{rep}") for i in range(2)]
                load_chunkT(Qs_d, sc * 512, qx, tp_ps, keep=qnat[2 * sc : 2 * sc + 2])
                if sc == 0:
                    load_weights_q()
                for p in range(PAIRS):
                    pr = proj_ps.tile([128, 512], f32, tag="proj")
                    for ec in range(4):
                        nc.tensor.matmul(
                            pr[:], wq[ec][:, p * 128 : (p + 1) * 128],
                            qx[ec // 2][:, (ec % 2) * 512 : (ec % 2) * 512 + 512],
                            start=(ec == 0), stop=(ec == 3),
                        )
                    nc.scalar.activation(
                        qT[p][:, sc * 512 : (sc + 1) * 512], pr[:], AF.Identity,
                        bias=bq_t[:, p : p + 1],
                    )
            # K -> kT pairs
            for sc in range(S // 512):
                kx = [xt_p.tile([128, 1024], f16, tag="xt", name=f"kx{sc}_{i}_{rep}") for i in range(2)]
                load_chunkT(Kf_d, sc * 512, kx, tp_ps)
                if sc == 0:
                    load_weights_k()
                elif sc == 2:
                    load_weights_rest()
                for p in range(PAIRS):
                    pr = proj_ps.tile([128, 512], f32, tag="proj")
                    for ec in range(4):
                        nc.tensor.matmul(
                            pr[:], wk[ec][:, p * 128 : (p + 1) * 128],
                            kx[ec // 2][:, (ec % 2) * 512 : (ec % 2) * 512 + 512],
                            start=(ec == 0), stop=(ec == 3),
                        )
                    nc.scalar.activation(
                        kT[p][:, sc * 512 : (sc + 1) * 512], pr[:], AF.Identity,
                        bias=bk_t[:, p : p + 1],
                    )
            # V -> v_aug (natural [seq, hd] with a ones column per head)
            for sc in range(S // 512):
                vx = [xt_p.tile([128, 1024], f16, tag="xt", name=f"vx{sc}_{i}_{rep}") for i in range(2)]
                load_chunkT(Vf_d, sc * 512, vx, tp_ps)
                for tl in range(4):
                    tt = sc * 4 + tl
                    pr = proj_ps.tile([128, 512], f32, tag="proj")
                    for ec in range(4):
                        nc.tensor.matmul(
                            pr[:],
                            vx[ec // 2][:, (ec % 2) * 512 + tl * 128 : (ec % 2) * 512 + (tl + 1) * 128],
                            wv[ec][:],
                            start=(ec == 0), stop=(ec == 3),
                        )
                    va3 = v_aug[tt][:].rearrange("p (h x) -> p h x", h=H, x=DK + 1)
                    pr3 = pr[:].rearrange("p (h d) -> p h d", h=H, d=DK)
                    nc.vector.tensor_copy(va3[:, :, 0:DK], pr3)
                    nc.vector.memset(va3[:, :, DK : DK + 1], 1.0)

        # bf_eff / gamma / beta broadcasts + exp-table warmup, in their own
        # small PSUM scope between the projection and attention phases
        with tc.tile_pool(name="psum_pre", bufs=1, space="PSUM") as pre_ps:
            emit_pre(pre_ps)

        # ---------- attention: software-pipelined scores/exp/PV ----------
        # Even head of pair lives on partitions 0-63, odd on 64-127 of the
        # qT/kT pair tiles -> the two K=64 scores matmuls of a pair run on
        # disjoint PE row-groups concurrently.
        with (
            tc.tile_pool(name="psum_sc", bufs=2, space="PSUM") as sc_ps_p,
            tc.tile_pool(name="psum_pv", bufs=2, space="PSUM") as pv_ps_p,
        ):
            pvs_all = [None] * PAIRS
            exs_all = [[None] * 16 for _ in range(PAIRS)]

            def emit_scores_exp(p, tt):
                scs = [
                    sc_ps_p.tile([128, SQ], f32, tag="sc", name=f"sc{p}_{tt}_{h}_{rep}")
                    for h in range(2)
                ]
                # interleave halves so adjacent matmuls sit on disjoint PE
                # row-groups (0-63 vs 64-127) and stream concurrently
                for qc in range(SQ // 512):
                    for half in range(2):
                        pb = 64 * half
                        nc.tensor.matmul(
                            scs[half][:, qc * 512 : (qc + 1) * 512],
                            kT[p][pb : pb + DK, tt * 128 : (tt + 1) * 128],
                            qT[p][pb : pb + DK, qc * 512 : (qc + 1) * 512],
                            start=True, stop=True,
                        )
                # half0 -> ScalarE true exp; half1 -> VectorE fast-exp.
                # At tt 0/8 (away from the pair boundary, where ScalarE does
                # the denominator copies) half1 also goes to ScalarE so the
                # DVE keeps headroom for the previous pair's norm.
                exa = exa_p.tile([128, SQ], f16, tag="exa", name=f"exa{p}_{tt}_{rep}")
                nc.scalar.activation(exa[:], scs[0][:], AF.Exp, scale=SCALE)
                if tt in (0, 8):
                    exd = exa_p.tile([128, SQ], f16, tag="exa", name=f"exd{p}_{tt}_{rep}")
                    nc.scalar.activation(exd[:], scs[1][:], AF.Exp, scale=SCALE)
                    exs_all[p][tt] = (exa, exd[:])
                else:
                    exd = exd_p.tile([128, SQ], i16, tag="exd", name=f"exd{p}_{tt}_{rep}")
                    nc.vector.tensor_scalar(
                        exd[:], scs[1][:], SCH_A, SCH_B, Alu.mult, Alu.add
                    )
                    exs_all[p][tt] = (exa, exd[:].bitcast(f16))

            def emit_pv(p, tt):
                exa, exd = exs_all[p][tt]
                pvs = pvs_all[p]
                for half, ex in ((0, exa[:]), (1, exd)):
                    h = 2 * p + half
                    for qc in range(SQ // 512):
                        nc.tensor.matmul(
                            pvs[half][:, qc * 512 : (qc + 1) * 512],
                            v_aug[tt][:, h * (DK + 1) : (h + 1) * (DK + 1)],
                            ex[:, qc * 512 : (qc + 1) * 512],
                            start=(tt == 0), stop=(tt == 15),
                        )
                exs_all[p][tt] = None

            def emit_norm(p):
                # normalize: ScalarE copies the raw denominator row to SBUF
                # (fp16), PE broadcasts it down 64 partitions, VectorE takes
                # the reciprocal on 64 lanes, then one mul per half into the
                # pair-stacked zT
                pvs = pvs_all[p]
                for half in range(2):
                    den = rd_p.tile([1, SQ], f16, tag="rd", name=f"rd{p}_{half}_{rep}")
                    nc.scalar.copy(den[:], pvs[half][DK : DK + 1, :])
                    rb = sc_ps_p.tile([128, SQ], f32, tag="sc", name=f"rb{p}_{half}_{rep}")
                    for qc in range(SQ // 512):
                        nc.tensor.matmul(
                            rb[0:DK, qc * 512 : (qc + 1) * 512],
                            ones_h[:, 0:DK],
                            den[:, qc * 512 : (qc + 1) * 512],
                            start=True, stop=True,
                        )
                    rb_sb = rd_p.tile([DK, SQ], f32, tag="rb", name=f"rb{p}_{half}_{rep}")
                    nc.vector.reciprocal_approx_fast(rb_sb[:], rb[0:DK, :])
                    with nc.allow_low_precision(reason="fp16 attention ctx"):
                        nc.vector.tensor_mul(
                            zT[p][64 * half : 64 * half + DK, :],
                            pvs[half][0:DK, :],
                            rb_sb[:],
                        )
                pvs_all[p] = None

            for g in range(PAIRS * 16 + 1):
                if g < PAIRS * 16:
                    p, tt = divmod(g, 16)
                    if tt == 0:
                        pvs_all[p] = [
                            pv_ps_p.tile(
                                [DK + 1, SQ], f32, tag="pv", name=f"pv{p}_{h}_{rep}"
                            )
                            for h in range(2)
                        ]
                    emit_scores_exp(p, tt)
                if g > 0:
                    pp, ptt = divmod(g - 1, 16)
                    emit_pv(pp, ptt)
                    if ptt == 15:
                        emit_norm(pp)

        # ---------- final linear + residual + LayerNorm ----------
        with tc.tile_pool(name="psum_f", bufs=4, space="PSUM") as f_ps_p:
            for qb in range(SQ // 128):
                f_ps = f_ps_p.tile([128, E], f32, tag="f")
                for p in range(PAIRS):
                    nc.tensor.matmul(
                        f_ps[:], zT[p][:, qb * 128 : (qb + 1) * 128], wfp[p][:],
                        start=(p == 0), stop=False,
                    )
                nc.tensor.matmul(
                    f_ps[:], ones_h[:, 0:128], bfe_sb[:],
                    start=False, stop=True,
                )
                qn = qnat[qb // 2][:, (qb % 2) * E : (qb % 2) * E + E].bitcast(f32)
                x = ln_p.tile([128, E], f32, tag="x")
                nm = st_p.tile([128, 1], f32, tag="nm")
                nc.vector.scalar_tensor_tensor(
                    x[:], f_ps[:], 1.0, qn,
                    mybir.AluOpType.mult, mybir.AluOpType.add,
                    accum_out=nm[:],
                )
                nc.vector.tensor_scalar_mul(nm[:], nm[:], -1.0 / E)
                # Square's tensor output is dead (only accum_out matters);
                # dump it into xn, which the Identity op fully overwrites below
                xn = ln_p.tile([128, E], f32, tag="xn")
                ss = st_p.tile([128, 1], f32, tag="ss")
                nc.scalar.activation(xn[:], x[:], AF.Square, accum_out=ss[:])
                # var = E[x^2] - mu^2; bias for sqrt = eps - mu^2
                vb = st_p.tile([128, 1], f32, tag="vb")
                nc.vector.scalar_tensor_tensor(
                    vb[:], nm[:], -1.0, nm[:],
                    mybir.AluOpType.mult, mybir.AluOpType.mult,
                )
                nc.vector.tensor_add(vb[:], vb[:], eps_t[:])
                sd = st_p.tile([128, 1], f32, tag="sd")
                nc.scalar.activation(
                    sd[:], ss[:], AF.Sqrt, bias=vb[:, 0:1], scale=1.0 / E
                )
                rstd = st_p.tile([128, 1], f32, tag="rstd")
                nc.vector.reciprocal(rstd[:], sd[:])
                nmr = st_p.tile([128, 1], f32, tag="nmr")
                nc.vector.tensor_mul(nmr[:], nm[:], rstd[:])
                nc.scalar.activation(
                    xn[:], x[:], AF.Identity, bias=nmr[:, 0:1], scale=rstd[:, 0:1]
                )
                # gamma on GpSimd (slow but mid-chain, overlaps other qbs);
                # beta on VectorE so the store isn't gated on a 1.3us Pool op
                nc.gpsimd.tensor_tensor(xn[:], xn[:], gab[:], mybir.AluOpType.mult)
                nc.vector.tensor_add(xn[:], xn[:], beb[:])
                nc.sync.dma_start(Out_d[qb * 128 : (qb + 1) * 128, :], xn[:])


def _get_program(repeat=1):
    key = f"nc{repeat}"
    if key not in _PROGRAM_CACHE:
        _PROGRAM_CACHE[key] = _build_program(repeat)
    return _PROGRAM_CACHE[key]


def _make_in_maps(Q, K, V, Wq, bq, Wk, bk, Wv, bv, Wf, bf, gamma, beta):
    f32 = np.float32

    f16 = np.float16

    def per_head_w(W):  # [H, E, DK] -> [E, H*DK], fp16
        return np.ascontiguousarray(W.transpose(1, 0, 2).reshape(E, HD), dtype=f16)

    def pair_bias(b, dt=f32):  # [H, DK] -> [128, PAIRS]; partition = (h%2)*64 + d
        return np.ascontiguousarray(
            b.reshape(PAIRS, 2, DK).transpose(1, 2, 0).reshape(128, PAIRS), dtype=dt
        )

    Wq_r, Wk_r, Wv_r = per_head_w(Wq), per_head_w(Wk), per_head_w(Wv)
    bq_r, bk_r = pair_bias(bq), pair_bias(bk)
    bv_r = pair_bias(bv, f16)
    Wf_c = np.ascontiguousarray(Wf, dtype=f16)
    bf_r = np.ascontiguousarray(bf.reshape(1, E), dtype=f32)
    ga_r = np.ascontiguousarray(gamma.reshape(1, E), dtype=f32)
    be_r = np.ascontiguousarray(beta.reshape(1, E), dtype=f32)

    in_maps = []
    for c in range(NCORES):
        b, qh = c // 2, c % 2
        in_maps.append(
            {
                "Qs": np.ascontiguousarray(Q[b, qh * SQ : (qh + 1) * SQ], dtype=f32),
                "Kf": np.ascontiguousarray(K[b], dtype=f32),
                "Vf": np.ascontiguousarray(V[b], dtype=f32),
                "Wq": Wq_r,
                "Wk": Wk_r,
                "Wv": Wv_r,
                "Wf": Wf_c,
                "bq_t": bq_r,
                "bk_t": bk_r,
                "bv_t": bv_r,
                "bf_r": bf_r,
                "gamma_r": ga_r,
                "beta_r": be_r,
                "ident": np.eye(128, dtype=f32),
            }
        )
    return in_maps


def run_spmd(in_maps, **kwargs):
    from concourse.bass_utils import run_bass_kernel_spmd

    nc = _get_program()
    return run_bass_kernel_spmd(nc, in_maps, list(range(NCORES)), **kwargs)


def kernel(**inputs) -> np.ndarray:
    in_maps = _make_in_maps(**inputs)
    res = run_spmd(in_maps)
    out = np.empty((B, S, E), np.float32)
    for c in range(NCORES):
        b, qh = c // 2, c % 2
        out[b, qh * SQ : (qh + 1) * SQ, :] = res.results[c]["Out"]
    return out


if __name__ == "__main__":
    import time

    t0 = time.time()
    _get_program()
    print(f"built ok in {time.time() - t0:.1f}s")
